# revision 37
# baseline (speedup 1.0000x reference)
"""DANet attention (PAM + CAM + fuse conv3x3 + BN + ReLU) on 8 TRN2 NeuronCores.

Sharding: core = 2*b + j handles sample b (of 4) and spatial band j (of 2).
Each band covers 34 rows of the 64-row image (32 output rows + 1 halo row on
each side; out-of-image halo rows are zero). PAM/CAM are computed for the
band's 2176 positions against the full 4096-position sample; the 3x3 fuse
conv runs on a zero-padded 34x66 layout; BN batch stats are combined across
all 8 cores with one tiny AllReduce per channel chunk.

PAM: the q bias is folded into q at projection time (the k-bias term is
constant per query and cancels in the softmax), so the energy contraction is
exactly K=32 and each m-quad runs as 4 concurrent row-tiled matmuls
(tile_position=(32i,0)) against 4-way partition-replicated k/q copies built
with SBUF-to-SBUF DMAs. exp(e) lands unnormalized in bf16; gamma_p*V^T
(host-prescaled) is applied directly in bf16 matmuls and the softmax
denominator (DVE slab adds + ones-matmul partition reduce) divides the
result once at the end via a broadcast reciprocal row.

CAM energy is computed from a host-provided x^T (no PE transposes).

Self-contained: hardcodes shapes B=4, C=256, H=W=64, RC=32.
"""

import numpy as np
import ml_dtypes

import concourse.bass as bass
import concourse.tile as tile
from concourse import bacc, mybir
from concourse.bass_utils import run_bass_kernel_spmd
F32 = mybir.dt.float32
F32R = mybir.dt.float32r
BF16 = mybir.dt.bfloat16
FP8 = mybir.dt.float8e4
AF = mybir.ActivationFunctionType
ALU = mybir.AluOpType
DR = mybir.MatmulPerfMode.DoubleRow

B, C, H, W = 4, 256, 64, 64
N = H * W            # 4096
RC = 32              # reduced channels for q/k
NB = 34 * W          # 2176 band positions (34 rows incl. halo/phantom rows)
PADW = W + 2         # 66
FLAT = 34 * PADW     # 2244 padded band slots
CCH = 2              # channel chunks of 128 (C = 256)
ICH = 4              # conv input-channel chunks of 128 (2C = 512)
MG = N // 128        # 32 m-chunks in PAM contraction
NQ = MG // 4         # 8 m-quads
BN_EPS = 1e-5

# PAM/CAM band tiles: (start, len) over the 2176 band positions
PAM_TILES = [(0, 512), (512, 512), (1024, 512), (1536, 512), (2048, 128)]
# conv output tiles: (padded row start, n rows); valid output rows are 1..32
CONV_ROWS = [(1, 7), (8, 7), (15, 7), (22, 7), (29, 4)]


def build(n_cores=8, stat_count=4 * N):
    """Build and compile the SPMD kernel graph. Returns compiled Bacc."""
    nc = bacc.Bacc("TRN2", target_bir_lowering=False, debug=False,
                   num_devices=n_cores)

    # ---- DRAM parameters (per core) ----
    x_full = nc.dram_tensor("x_full", [CCH, 128, N], BF16, kind="ExternalInput")
    xT_d = nc.dram_tensor("xT", [128, MG, C], BF16, kind="ExternalInput")
    x8_d = nc.dram_tensor("x8", [128, 2, N], FP8, kind="ExternalInput")
    x_band = nc.dram_tensor("x_band", [CCH, 128, NB], BF16, kind="ExternalInput")
    xb8_d = nc.dram_tensor("xb8", [128, 2, NB], FP8, kind="ExternalInput")
    qwT_d = nc.dram_tensor("qwT", [C, 128], BF16, kind="ExternalInput")
    kwT_d = nc.dram_tensor("kwT", [C, RC], BF16, kind="ExternalInput")
    qb_d = nc.dram_tensor("qb", [128, 1], F32, kind="ExternalInput")
    # v^T weights as fp8 pairs [cin, ct, c], pre-scaled by 64*gamma_p
    vwT_d = nc.dram_tensor("vwT", [128, 2 * C], FP8, kind="ExternalInput")
    vbg_d = nc.dram_tensor("vbg", [128, CCH], F32, kind="ExternalInput")
    fw_d = nc.dram_tensor("fw", [ICH, 128, 9 * 2 * 128], BF16, kind="ExternalInput")
    gc_d = nc.dram_tensor("gc", [128, 1], F32, kind="ExternalInput")   # 128*gamma_c
    mf_d = nc.dram_tensor("mf", [128, 1], F32, kind="ExternalInput")   # first-row mask
    ml_d = nc.dram_tensor("ml", [128, 1], F32, kind="ExternalInput")   # last-row mask
    bnsc_d = nc.dram_tensor("bnsc", [128, CCH], F32, kind="ExternalInput")
    ones_m1f_d = nc.dram_tensor("ones_m1f", [1, 128], F32, kind="ExternalInput")
    ones_colb_d = nc.dram_tensor("ones_colb", [128, 1], BF16, kind="ExternalInput")
    ident_d = nc.dram_tensor("ident", [128, 128], F32R, kind="ExternalInput")
    bnbi_d = nc.dram_tensor("bnbi", [128, CCH], F32, kind="ExternalInput")
    y_out = nc.dram_tensor("y_out", [CCH, 128, 32 * W], F32, kind="ExternalOutput")
    st_loc = [nc.dram_tensor(f"st_loc{i}", [128, 2], F32) for i in range(CCH)]
    st_glob = [nc.dram_tensor(f"st_glob{i}", [128, 2], F32, addr_space="Shared")
               for i in range(CCH)]

    with tile.TileContext(nc) as tc:
        with tc.tile_pool(name="persist", bufs=1) as pp, \
             tc.tile_pool(name="psum", bufs=1, space="PSUM") as psp:

            # ---- persistent SBUF tiles ----
            # 4-way row-replicated k with STRIDED quad mapping: partition
            # 32i+r, quad Q, col m holds k[r, (8i+Q)*128+m]. This makes each
            # k-projection tile (4 consecutive chunks, constant i=t//2) land
            # with a single contiguous scalar copy.
            k_sb4 = pp.tile([128, NQ, 128], BF16)
            # 4-way partition-replicated q' (= Wq x + qb)
            q_sb4 = pp.tile([128, NB], BF16)
            vT = pp.tile([128, MG, C], BF16)      # gamma_p * v^T, bf16
            xb = pp.tile([128, CCH, NB], BF16)
            xb8 = pp.tile([128, 2, NB], FP8)
            # projection weights replicated 4x along the output dim so the
            # psum results land directly in the partition-replicated layouts
            qwT = pp.tile([128, CCH, 128], BF16)
            kwT = pp.tile([128, CCH, RC], BF16)
            qb = pp.tile([128, 1], F32)
            vwT = pp.tile([128, 2, C], FP8)
            vbg = pp.tile([128, CCH], F32)
            ones_m1f = pp.tile([1, 128], F32)
            ones_colb = pp.tile([128, 1], BF16)
            ident = pp.tile([128, 128], F32R)
            gc = pp.tile([128, 1], F32)
            mf = pp.tile([128, 1], F32)
            ml = pp.tile([128, 1], F32)
            bnsc = pp.tile([128, CCH], F32)
            bnbi = pp.tile([128, CCH], F32)
            ec_sb = [pp.tile([128, C], F32, name=f"ec_sb{i}") for i in range(CCH)]
            attn_c = [pp.tile([128, C], F32R, name=f"attn_c{i}") for i in range(CCH)]
            # (128*gamma_c)*attn_c^T in fp8 pairs: [d-part, ct, dc, c]
            attn_cT = pp.tile([128, 2, 2, 128], FP8)
            fw = [pp.tile([128, 9 * 2 * 128], BF16, name=f"fw{i}") for i in range(ICH)]
            feats = [pp.tile([128, FLAT + 2], BF16, name=f"feats{i}") for i in range(ICH)]
            y_sb = [pp.tile([128, 32 * PADW], F32, name=f"y_sb{i}") for i in range(CCH)]
            s_acc2 = pp.tile([128, 2, 512], BF16)
            st_sb = pp.tile([128, 2 * CCH], F32)
            stg = pp.tile([128, 2 * CCH], F32)
            epsc = pp.tile([128, 1], F32)

            nc.gpsimd.memset(epsc, BN_EPS)
            for i in range(ICH):
                nc.gpsimd.memset(feats[i], 0.0)

            # ================= phase 1: xf-derived tensors =================
            with tc.tile_pool(name="early", bufs=1) as ep:
                xf = ep.tile([128, CCH, N], BF16, name="xf")
                xf8 = ep.tile([128, 2, N], FP8, name="xf8")
                xT = ep.tile([128, MG, C], BF16, name="xT")
                # stream x layouts in per-quad groups so the g-loop can start
                # as soon as the first slabs land
                nc.sync.dma_start(out=kwT[:, 0, :], in_=kwT_d.ap()[0:128, :])
                nc.sync.dma_start(out=kwT[:, 1, :], in_=kwT_d.ap()[128:256, :])
                nc.sync.dma_start(out=qb, in_=qb_d.ap())
                nc.sync.dma_start(out=vwT, in_=vwT_d.ap())
                for t in range(NQ):
                    for i in range(CCH):
                        nc.sync.dma_start(
                            out=xf[:, i, t * 512:(t + 1) * 512],
                            in_=x_full.ap()[i][:, t * 512:(t + 1) * 512])
                    nc.sync.dma_start(out=xf8[:, :, t * 512:(t + 1) * 512],
                                      in_=x8_d.ap()[:, :, t * 512:(t + 1) * 512])
                    if t == 0:
                        nc.sync.dma_start(out=qwT[:, 0, :], in_=qwT_d.ap()[0:128, :])
                        nc.sync.dma_start(out=qwT[:, 1, :],
                                          in_=qwT_d.ap()[128:256, :])
                    if t < len(PAM_TILES):
                        (sx, lnx) = PAM_TILES[t]
                        for i in range(CCH):
                            nc.sync.dma_start(out=xb[:, i, sx:sx + lnx],
                                              in_=x_band.ap()[i][:, sx:sx + lnx])
                for t in range(NQ):
                    nc.sync.dma_start(out=xT[:, 4 * t:4 * t + 4, :],
                                      in_=xT_d.ap()[:, 4 * t:4 * t + 4, :])
                nc.sync.dma_start(out=xb8, in_=xb8_d.ap())
                nc.sync.dma_start(out=ident, in_=ident_d.ap())
                for t, d in ((gc, gc_d), (mf, mf_d), (ml, ml_d),
                             (bnsc, bnsc_d), (bnbi, bnbi_d), (vbg, vbg_d),
                             (ones_m1f, ones_m1f_d), (ones_colb, ones_colb_d)):
                    nc.sync.dma_start(out=t, in_=d.ap())

                # CAM energy accumulators: one PSUM bank per ct chain (two
                # interleaved accumulation groups must not share a bank)
                ecp = [psp.tile([128, C], F32, tag="small", bufs=2, name=f"ecp{i}")
                       for i in range(CCH)]
                # Interleaved emission: k-proj (8 tiles), q-proj (5), vT (32),
                # CAM energy from xT (32) — keeps the PE stream dense and real
                # (no transpose-mode ops, which HAM ignores).
                kq4 = [None]
                for g in range(MG):
                    if g < NQ + len(PAM_TILES):
                        # 4-deep projection psum: one [128,4,512] etq-slot
                        # allocation hosts 4 units in separate banks
                        if g % 4 == 0:
                            kq4[0] = psp.tile([128, 4, 512], F32, tag="etq",
                                              bufs=1, name="kq4")
                        jslot = g % 4
                    if g < NQ:
                        t = g
                        for ct in range(CCH):
                            nc.tensor.matmul(kq4[0][0:RC, jslot, :],
                                             lhsT=kwT[:, ct, :],
                                             rhs=xf[:, ct, t * 512:(t + 1) * 512],
                                             start=(ct == 0), stop=(ct == CCH - 1))
                        # chunks 4t..4t+3 = {8i+Q : i=t//2, Q=4(t%2)..+4}:
                        # one contiguous copy into the strided quad layout
                        nc.scalar.activation(
                            k_sb4[32 * (t // 2):32 * (t // 2) + 32,
                                  4 * (t % 2):4 * (t % 2) + 4, :],
                            kq4[0][0:RC, jslot, :].rearrange(
                                "p (q m) -> p q m", m=128), AF.Copy)
                    if NQ <= g < NQ + len(PAM_TILES):
                        (sq_, ln) = PAM_TILES[g - NQ]
                        for ct in range(CCH):
                            nc.tensor.matmul(kq4[0][:, jslot, 0:ln],
                                             lhsT=qwT[:, ct, :],
                                             rhs=xb[:, ct, sq_:sq_ + ln],
                                             start=(ct == 0), stop=(ct == CCH - 1))
                        # fold the q bias here: q' = Wq x + qb
                        nc.scalar.activation(q_sb4[:, sq_:sq_ + ln],
                                             kq4[0][:, jslot, 0:ln],
                                             AF.Identity, bias=qb)
                    # vT chunk g: one fp8 DoubleRow matmul (K=256); the 1/64
                    # undoes the 64*gamma_p host prescale (gamma_p stays in)
                    vp = psp.tile([128, C], F32, tag="u", bufs=2, name="vp")
                    nc.tensor.matmul(vp, lhsT=xf8[:, :, g * 128:(g + 1) * 128],
                                     rhs=vwT, perf_mode=DR, start=True, stop=True)
                    nc.vector.tensor_scalar_mul(vT[:, g, :], vp, 1.0 / 64.0)
                    # CAM energy accumulation straight from host x^T
                    for ct in range(CCH):
                        nc.tensor.matmul(ecp[ct],
                                         lhsT=xT[:, g, ct * 128:(ct + 1) * 128],
                                         rhs=xT[:, g, :],
                                         start=(g == 0), stop=(g == MG - 1))
                for ct in range(CCH):
                    nc.vector.tensor_copy(ec_sb[ct], ecp[ct])

            # fuse-conv weights: needed only in phase 4; load after the x DMAs
            for i in range(ICH):
                nc.sync.dma_start(out=fw[i], in_=fw_d.ap()[i])

            # ============ phases 2-5 use the late pool (reuses early space) ====
            with tc.tile_pool(name="late", bufs=1) as lp:
                # ---- phase 2: CAM softmax (cheap; the apply is emitted after
                # PAM so PAM's energy stream starts as early as possible) ----
                for ct in range(CCH):
                    emin = lp.tile([128, 1], F32, tag="bn_t", bufs=8, name="emin")
                    esum = lp.tile([128, 1], F32, tag="bn_t", bufs=8, name="esum")
                    erec = lp.tile([128, 1], F32, tag="bn_t", bufs=8, name="erec")
                    ea = lp.tile([128, C], F32, tag="ea", bufs=2, name="ea")
                    nc.vector.tensor_reduce(emin, ec_sb[ct], axis=mybir.AxisListType.X,
                                            op=ALU.min)
                    # exp(min - e), with row-sum accumulated on the fly
                    nc.scalar.activation(ea, ec_sb[ct], AF.Exp, bias=emin, scale=-1.0,
                                         accum_out=esum)
                    nc.vector.reciprocal(erec, esum)
                    nc.vector.tensor_scalar_mul(attn_c[ct], ea, erec)

                # ---- phase 3: PAM ----
                # per m-quad: 4 concurrent row-tiled energy matmuls -> 2
                # double-bank exps -> 2 slab adds; applies trail one quad.
                def pam_tile(ti):
                    (s, ln) = PAM_TILES[ti]
                    u = [psp.tile([128, 512], F32, tag="u", bufs=2, name=f"u{i}")
                         for i in range(CCH)]
                    eq = [None] * NQ

                    def applies(Q):
                        # bank i of quad Q holds chunk 8i+Q (strided mapping)
                        for i in range(4):
                            for ct in range(CCH):
                                nc.tensor.matmul(
                                    u[ct][:, 0:ln],
                                    lhsT=vT[:, 8 * i + Q, ct * 128:(ct + 1) * 128],
                                    rhs=eq[Q][:, i, 0:ln],
                                    start=(Q == 0 and i == 0),
                                    stop=(Q == NQ - 1 and i == 3))

                    for Q in range(NQ):
                        etq = psp.tile([128, 4, 512], F32, tag="etq", bufs=1,
                                       name="etq")
                        for i in range(4):
                            nc.tensor.matmul(
                                etq[:, i, 0:ln],
                                lhsT=k_sb4[32 * i:32 * (i + 1), Q, :],
                                rhs=q_sb4[32 * i:32 * (i + 1), s:s + ln],
                                start=True, stop=True, tile_position=(32 * i, 0))
                        eq[Q] = lp.tile([128, 4, 512], BF16, tag="e_q", bufs=3,
                                        name="e_q")
                        for j in range(2):
                            nc.scalar.activation(eq[Q][:, 2 * j:2 * j + 2, 0:ln],
                                                 etq[:, 2 * j:2 * j + 2, 0:ln],
                                                 AF.Exp)
                        # denominator slab accumulation (bf16, 1024 elem ops)
                        for j in range(2):
                            if Q == 0 and j == 0:
                                nc.vector.tensor_copy(s_acc2[:, :, 0:ln],
                                                      eq[0][:, 0:2, 0:ln])
                            else:
                                nc.vector.tensor_add(s_acc2[:, :, 0:ln],
                                                     s_acc2[:, :, 0:ln],
                                                     eq[Q][:, 2 * j:2 * j + 2, 0:ln])
                        if Q >= 1:
                            applies(Q - 1)
                    applies(NQ - 1)

                    # denominator finish: fold pair, partition-reduce, recip,
                    # broadcast back to 128 partitions
                    nc.vector.tensor_add(s_acc2[:, 0, 0:ln], s_acc2[:, 0, 0:ln],
                                         s_acc2[:, 1, 0:ln])
                    ssum = psp.tile([1, 512], F32, tag="small", bufs=2, name="ssum")
                    nc.tensor.matmul(ssum[:, 0:ln], lhsT=ones_colb,
                                     rhs=s_acc2[:, 0, 0:ln], start=True, stop=True)
                    rrow = lp.tile([1, 512], F32, tag="ssb", bufs=2, name="rrow")
                    nc.vector.reciprocal_approx_fast(rrow[:, 0:ln], ssum[:, 0:ln])
                    rbq = psp.tile([128, 512], F32, tag="small", bufs=2, name="rbq")
                    nc.tensor.matmul(rbq[:, 0:ln], lhsT=ones_m1f, rhs=rrow[:, 0:ln],
                                     start=True, stop=True)
                    rb_sb = lp.tile([128, 512], BF16, tag="rb_sb", bufs=2,
                                    name="rb_sb")
                    nc.scalar.activation(rb_sb[:, 0:ln], rbq[:, 0:ln], AF.Copy)
                    last_rb[0] = rb_sb

                    r0, nr = s // W, ln // W
                    for ct in range(CCH):
                        # m1 = (gamma_p*pam_raw) * (1/denom)
                        m1 = lp.tile([128, 512], BF16, tag="m1", bufs=2, name="m1")
                        nc.vector.tensor_mul(m1[:, 0:ln], u[ct][:, 0:ln],
                                             rb_sb[:, 0:ln])
                        # position = m1 + gamma_p*vb + x_band -> feats[0..1]
                        fdst = feats[ct][:, 2:2 + 34 * PADW] \
                            .rearrange("p (r w) -> p r w", w=PADW)[:, r0:r0 + nr, 0:W]
                        fsrc = m1[:, 0:ln].rearrange("p (r w) -> p r w", w=W)
                        fx = xb[:, ct, s:s + ln].rearrange("p (r w) -> p r w", w=W)
                        nc.vector.scalar_tensor_tensor(fdst, fsrc, vbg[:, ct:ct + 1],
                                                       fx, op0=ALU.add, op1=ALU.add)

                last_rb = [None]
                for ti in range(len(PAM_TILES)):
                    pam_tile(ti)

                # ---- CAM transpose + apply (PE filler between PAM and conv) --
                # transpose attn_c -> (128*gamma_c)*attn_c^T fp8 [d, ct, dc, c]
                for dc in range(CCH):
                    for ct in range(CCH):
                        tp2 = psp.tile([128, 128], F32R, tag="small", bufs=2, name="tp2")
                        nc.tensor.transpose(tp2, attn_c[ct][:, dc * 128:(dc + 1) * 128],
                                            ident)
                        nc.vector.tensor_scalar_mul(attn_cT[:, ct, dc, :], tp2, gc)
                # cam: channel = (128*gc*cam)/128 + x_band -> feats[2..3]
                for ct in range(CCH):
                    for (s, ln) in PAM_TILES:
                        cp = psp.tile([128, 512], F32, tag="u", bufs=2, name="cp")
                        nc.tensor.matmul(cp[:, 0:ln], lhsT=attn_cT[:, ct, :, :],
                                         rhs=xb8[:, :, s:s + ln], perf_mode=DR,
                                         start=True, stop=True)
                        r0, nr = s // W, ln // W
                        fdst = feats[CCH + ct][:, 2:2 + 34 * PADW] \
                            .rearrange("p (r w) -> p r w", w=PADW)[:, r0:r0 + nr, 0:W]
                        fsrc_cam = cp[:, 0:ln].rearrange("p (r w) -> p r w", w=W)
                        fx = xb[:, ct, s:s + ln].rearrange("p (r w) -> p r w", w=W)
                        nc.vector.scalar_tensor_tensor(fdst, fsrc_cam, 1.0 / 128.0,
                                                       fx, op0=ALU.mult, op1=ALU.add)

                # phantom halo rows: multiply feats row 0 by mf, row 33 by ml
                for i in range(ICH):
                    fv = feats[i][:, 2:2 + 34 * PADW].rearrange("p (r w) -> p r w",
                                                                w=PADW)
                    nc.vector.tensor_scalar_mul(fv[:, 0:1, 0:W], fv[:, 0:1, 0:W], mf)
                    nc.vector.tensor_scalar_mul(fv[:, 33:34, 0:W], fv[:, 33:34, 0:W], ml)

                # preload the Sqrt activation table between PAM's exps and the
                # BN finalize; the data dependency on the last PAM tile's
                # reciprocal row keeps the scheduler from hoisting it into the
                # middle of the exp stream (which would thrash table sets)
                sqd = lp.tile([128, 1], F32, tag="bn_t", bufs=8, name="sqd")
                nc.scalar.activation(sqd, last_rb[0][:, 0:1], AF.Sqrt)

                # ---- phase 4: conv3x3 + incremental BN stats + per-ot AllReduce ----
                sparts = [[lp.tile([128, len(CONV_ROWS), 7], F32, tag="sparts",
                                   bufs=4, name=f"sparts{o}{i}") for i in range(2)]
                          for o in range(CCH)]
                for o in range(CCH):
                    nc.vector.memset(sparts[o][0], 0.0)
                    nc.vector.memset(sparts[o][1], 0.0)
                for ot in range(CCH):
                    for ci, (r0, nr) in enumerate(CONV_ROWS):
                        s, ln = r0 * PADW, nr * PADW
                        yp = psp.tile([128, 512], F32, tag="u", bufs=2, name="yp")
                        first = True
                        for tap in range(9):
                            off = (tap // 3 - 1) * PADW + (tap % 3 - 1)
                            o = 1 + s + off
                            for ic in range(ICH):
                                src = feats[ic][:, o:o + ln]
                                nc.tensor.matmul(
                                    yp[:, 0:ln],
                                    lhsT=fw[ic][:, (tap * 2 + ot) * 128:
                                                (tap * 2 + ot + 1) * 128],
                                    rhs=src,
                                    start=first, stop=(tap == 8 and ic == ICH - 1))
                                first = False
                        nc.vector.tensor_copy(y_sb[ot][:, s - PADW:s - PADW + ln],
                                              yp[:, 0:ln])
                        # incremental stats on the valid 64 columns of each row
                        yv = y_sb[ot].rearrange("p (r w) -> p r w", w=PADW)[
                            :, r0 - 1:r0 - 1 + nr, 1:65]
                        nc.vector.tensor_reduce(sparts[ot][0][:, ci, 0:nr], yv,
                                                axis=mybir.AxisListType.X, op=ALU.add)
                        sq = lp.tile([128, 7 * 64], F32, tag="sq", bufs=1, name="sq")
                        sqv = sq[:, 0:nr * 64].rearrange("p (r w) -> p r w", w=64)
                        nc.vector.scalar_tensor_tensor(
                            sqv, yv, 1.0, yv, op0=ALU.mult, op1=ALU.mult,
                            accum_out=sparts[ot][1].rearrange(
                                "p a b -> p (a b)")[:, 7 * ci:7 * ci + 1])
                    for i in range(2):
                        nc.vector.tensor_reduce(
                            st_sb[:, 2 * ot + i:2 * ot + i + 1],
                            sparts[ot][i].rearrange("p a b -> p (a b)"),
                            axis=mybir.AxisListType.X, op=ALU.add)
                    # keep the gpsimd sequencer warm so the collective
                    # dispatch doesn't pay a wake-up latency
                    gwarm = lp.tile([128, 1], F32, tag="bn_t", bufs=8,
                                    name="gwarm")
                    nc.gpsimd.memset(gwarm, 0.0)
                    # per-ot AllReduce: ot=0's hides under ot=1's conv
                    nc.sync.dma_start(out=st_loc[ot].ap(),
                                      in_=st_sb[:, 2 * ot:2 * ot + 2])
                    if n_cores > 1:
                        nc.gpsimd.collective_compute(
                            "AllReduce", ALU.add,
                            replica_groups=[list(range(n_cores))],
                            ins=[st_loc[ot].ap()], outs=[st_glob[ot].ap()])
                    else:
                        nc.gpsimd.dma_start(out=st_glob[ot].ap(),
                                            in_=st_loc[ot].ap())
                    nc.sync.dma_start(out=stg[:, 2 * ot:2 * ot + 2],
                                      in_=st_glob[ot].ap())

                # per-channel scale' = bn_scale * rstd ; bias' = bn_bias - mean*scale'
                inv_n = 1.0 / float(stat_count)
                for ot in range(CCH):
                    mean = lp.tile([128, 1], F32, tag="bn_t", bufs=8, name="mean")
                    msq = lp.tile([128, 1], F32, tag="bn_t", bufs=8, name="msq")
                    var = lp.tile([128, 1], F32, tag="bn_t", bufs=8, name="var")
                    m2 = lp.tile([128, 1], F32, tag="bn_t", bufs=8, name="m2")
                    std = lp.tile([128, 1], F32, tag="bn_t", bufs=8, name="std")
                    rstd = lp.tile([128, 1], F32, tag="bn_t", bufs=8, name="rstd")
                    sc2 = lp.tile([128, 1], F32, tag="bn_t", bufs=8, name="sc2")
                    bi2 = lp.tile([128, 1], F32, tag="bn_t", bufs=8, name="bi2")
                    t0 = lp.tile([128, 1], F32, tag="bn_t", bufs=8, name="t0")
                    nc.vector.tensor_scalar_mul(mean, stg[:, 2 * ot:2 * ot + 1], inv_n)
                    nc.vector.tensor_scalar_mul(msq, stg[:, 2 * ot + 1:2 * ot + 2], inv_n)
                    nc.vector.tensor_mul(m2, mean, mean)
                    nc.vector.tensor_sub(var, msq, m2)
                    nc.scalar.activation(std, var, AF.Sqrt, bias=epsc)
                    nc.vector.reciprocal(rstd, std)
                    nc.vector.tensor_mul(sc2, bnsc[:, ot:ot + 1], rstd)
                    nc.vector.tensor_mul(t0, mean, sc2)
                    nc.vector.tensor_sub(bi2, bnbi[:, ot:ot + 1], t0)
                    # y = relu(y*scale' + bias') on valid cols, then store
                    for hh in range(2):
                        stage = lp.tile([128, 16 * W], F32, tag="stage", bufs=2,
                                        name="stage")
                        ysrc = y_sb[ot].rearrange("p (r w) -> p r w", w=PADW)[
                            :, 16 * hh:16 * hh + 16, 1:65]
                        sview = stage.rearrange("p (r w) -> p r w", w=W)
                        if hh == 0:
                            nc.scalar.activation(sview, ysrc, AF.Relu,
                                                 bias=bi2, scale=sc2)
                        else:
                            nc.vector.tensor_scalar(sview, ysrc, sc2, bi2,
                                                    op0=ALU.mult, op1=ALU.add)
                            nc.vector.tensor_scalar_max(sview, sview, 0.0)
                        nc.sync.dma_start(
                            out=y_out.ap()[ot][:, 16 * W * hh:16 * W * (hh + 1)],
                            in_=stage)

    nc.compile()
    return nc


_CACHE = {}


def _get_nc(n_cores=8, stat_count=4 * N):
    key = (n_cores, stat_count)
    if key not in _CACHE:
        _CACHE[key] = build(n_cores, stat_count)
    return _CACHE[key]


def make_in_maps(x, q_w, q_b, k_w, k_b, v_w, v_b, gamma_p, gamma_c,
                 fuse_w, bn_scale, bn_bias, cores=8):
    f4 = np.float32
    bf = ml_dtypes.bfloat16
    f8 = ml_dtypes.float8_e4m3
    shared = {}
    gp_v = float(np.asarray(gamma_p, f4).ravel()[0])
    gc_v = float(np.asarray(gamma_c, f4).ravel()[0])
    # q weights tiled 4x along the output dim (partition-replicated psum
    # layout for the row-tiled energy matmuls); k scatters via copies instead
    shared["kwT"] = np.ascontiguousarray(np.asarray(k_w, f4).T).astype(bf)
    shared["qwT"] = np.ascontiguousarray(
        np.tile(np.asarray(q_w, f4).T, (1, 4))).astype(bf)
    shared["qb"] = np.tile(np.asarray(q_b, f4), 4).reshape(128, 1)
    # vwT fp8 pairs [cin-part, ct, c], scaled by 64*gamma_p (64 undone on
    # device, gamma_p stays folded into v^T)
    vwT = np.asarray(v_w, f4).T.reshape(2, 128, C).transpose(1, 0, 2)
    shared["vwT"] = np.ascontiguousarray(
        (vwT * (64.0 * gp_v)).reshape(128, 2 * C)).astype(f8)
    # gamma_p * v_b, laid out [128, ct] for the per-partition feats add
    shared["vbg"] = np.ascontiguousarray(
        (np.asarray(v_b, f4) * gp_v).reshape(CCH, 128).T)
    # fuse_w [256, 512, 3, 3] -> [ic, i, tap*2*128 + ot*128 + o] in bf16
    fwr = np.asarray(fuse_w, f4).reshape(CCH, 128, ICH, 128, 3, 3)
    fwt = np.ascontiguousarray(fwr.transpose(2, 3, 4, 5, 0, 1))  # ic,i,kh,kw,ot,o
    shared["fw"] = fwt.reshape(ICH, 128, 9 * 2 * 128).astype(bf)
    # CAM: attn_cT is scaled by 128*gamma_c on device (fp8 range use); the
    # apply multiplies cp by 1/128
    shared["gc"] = np.full((128, 1), 128.0 * gc_v, f4)
    shared["bnsc"] = np.ascontiguousarray(np.asarray(bn_scale, f4).reshape(CCH, 128).T)
    shared["bnbi"] = np.ascontiguousarray(np.asarray(bn_bias, f4).reshape(CCH, 128).T)
    shared["ones_m1f"] = np.ones((1, 128), f4)
    shared["ones_colb"] = np.ones((128, 1), bf)
    shared["ident"] = np.eye(128, dtype=f4)

    xs = np.asarray(x, f4).reshape(B, C, N)
    zrow = np.zeros((C, W), f4)
    in_maps = []
    for core in range(cores):
        b, j = core // 2, core % 2
        xf = xs[b]
        if j == 0:
            band = np.concatenate([zrow, xf[:, 0:33 * W]], axis=1)
            mfv, mlv = 0.0, 1.0
        else:
            band = np.concatenate([xf[:, 31 * W:], zrow], axis=1)
            mfv, mlv = 1.0, 0.0
        m = dict(shared)
        m["x_full"] = np.ascontiguousarray(xf.reshape(CCH, 128, N)).astype(bf)
        # xT[p, g, c] = x[c, 128g+p]
        m["xT"] = np.ascontiguousarray(
            xf.reshape(C, MG, 128).transpose(2, 1, 0)).astype(bf)
        m["x8"] = np.ascontiguousarray(
            xf.reshape(2, 128, N).transpose(1, 0, 2)).astype(f8)
        m["x_band"] = np.ascontiguousarray(band.reshape(CCH, 128, NB)).astype(bf)
        m["xb8"] = np.ascontiguousarray(
            band.reshape(2, 128, NB).transpose(1, 0, 2)).astype(f8)
        m["mf"] = np.full((128, 1), mfv, f4)
        m["ml"] = np.full((128, 1), mlv, f4)
        in_maps.append(m)
    return in_maps


def kernel(**inputs):
    nc = _get_nc(8)
    in_maps = make_in_maps(**inputs)
    r = run_bass_kernel_spmd(nc, in_maps, core_ids=list(range(8)))
    out = np.empty((B, C, H, W), np.float32)
    for core in range(8):
        b, j = core // 2, core % 2
        y2 = r.results[core]["y_out"]  # [2, 128, 2048]
        out[b, 0:128, 32 * j:32 * j + 32, :] = y2[0].reshape(128, 32, W)
        out[b, 128:256, 32 * j:32 * j + 32, :] = y2[1].reshape(128, 32, W)
    return out
